# revision 1
# baseline (speedup 1.0000x reference)
"""BERT self-attention (B=8, S=1024, H=1024, 16 heads) on 8 TRN2 NeuronCores.

Sharding: data-parallel over batch — core i computes batch element i end to
end (QKV projections + attention), no collectives.

Per-core dataflow (all matmuls in float32r on the PE array):
  X^T        via PE transposes of the loaded hidden_states tile
  Q^T, K^T   = Wq/Wk col-block^T-free matmuls against X^T  (layout [H', S])
  V          = X @ Wv in natural layout, scattered to per-head slices with an
               appended ones-column (V' = [V_h | 1])
  scores^T   = K_h @ Q_h^T per head ([key, query] layout, contraction=64)
  P = exp(scores^T/8 + mask)  on ScalarE straight out of PSUM (scale+bias fused)
  ctx'^T     = V'^T @ P  -> rows 0..63 = ctx^T, row 64 = softmax denominator
  ctx        = PE transpose of ctx'^T, rows scaled by 1/denominator on DVE
"""

import numpy as np

import concourse.bass as bass
import concourse.mybir as mybir
import concourse.tile as tile
from concourse.bass_utils import run_bass_kernel_spmd
from concourse.masks import make_identity
from concourse.vector_clock import ScopedClock

S = 1024
H = 1024
NH = 16
HD = 64
P = 128
NS = S // P  # s tiles
NK = H // P  # contraction tiles
NHP = NH // 2  # head pairs (one 128-partition tile of Q^T/K^T per pair)
VW = HD + 1  # V' width per head (extra ones column)
F32 = mybir.dt.float32
F32R = mybir.dt.float32r
N_CORES = 8

_DRAIN_WAIT_CHUNK = 1
_patched = False


def _patch_tile_drain():
    """The walrus build in this container rejects instructions carrying more
    than a few sem waits; TileContext's tail drain waits on every live sem in
    one instruction. Split it into one drain per wait."""
    global _patched
    if _patched:
        return
    _patched = True

    def _drain_and_barrier(self, tick_clock, wait_clock):
        drain_inst = self.nc.sync.drain()
        wait_clock.add_sem_waits(
            drain_inst.ins, ScopedClock({None: tick_clock.global_clock})
        )
        si = drain_inst.ins.sync_info
        if si is not None and len(si.on_wait) > _DRAIN_WAIT_CHUNK:
            waits = list(si.on_wait)
            si.on_wait = waits[:_DRAIN_WAIT_CHUNK]
            drain_inst.ins.sync_info = si
            for i in range(_DRAIN_WAIT_CHUNK, len(waits), _DRAIN_WAIT_CHUNK):
                extra = self.nc.sync.drain()
                extra.ins.sync_info = mybir.SyncInfo(
                    on_wait=waits[i : i + _DRAIN_WAIT_CHUNK], on_update=[]
                )
        self.nc.all_engine_barrier()
        popped = self.nc._tile_sem_poison_stack.pop()
        assert popped is self._sem_poison
        self.nc.clear_and_free_semaphores(list(self.sems.allocated().values()))
        self.nc.all_engine_barrier()

    tile.TileContext._drain_and_barrier = _drain_and_barrier


def _r(ap):
    return ap.bitcast(F32R)


def build_consts(nc, tc, ctx, mask, bq, bk, bv):
    """Constant tiles shared by all reps: identity, mask/bias layouts, ones."""
    singles = ctx.enter_context(tc.tile_pool(name="singles", bufs=1))
    ident = singles.tile([P, P], F32)
    make_identity(nc, ident)
    mask_t = singles.tile([P, NK], F32)
    nc.sync.dma_start(out=mask_t, in_=mask.rearrange("(t p) -> p t", p=P))
    bq_t = singles.tile([P, NHP], F32)
    nc.sync.dma_start(out=bq_t, in_=bq.rearrange("(t p) -> p t", p=P))
    bk_t = singles.tile([P, NHP], F32)
    nc.sync.dma_start(out=bk_t, in_=bk.rearrange("(t p) -> p t", p=P))
    ones_col = singles.tile([P, NH, 1], F32)
    nc.vector.memset(ones_col, 1.0)
    bv_b = singles.tile([P, H], F32)
    nc.gpsimd.dma_start(
        out=bv_b,
        in_=bass.AP(tensor=bv.tensor, offset=bv.offset, ap=[[0, P], bv.ap[0]]),
    )
    return ident, mask_t, bq_t, bk_t, ones_col, bv_b


def build_kernel(nc, tc, consts, hs, mask, wq, bq, wk, bk, wv, bv, out, phases=("x", "v", "qk", "attn")):
    from contextlib import ExitStack

    Exp = mybir.ActivationFunctionType.Exp
    ident, mask_t, bq_t, bk_t, ones_col, bv_b = consts

    ctx = ExitStack()
    with ctx:
        xt_pool = ctx.enter_context(tc.tile_pool(name="xt", bufs=NK))
        vp_pool = ctx.enter_context(tc.tile_pool(name="vp", bufs=NS))
        qk_pool = ctx.enter_context(tc.tile_pool(name="qk", bufs=2))
        wcol_pool = ctx.enter_context(tc.tile_pool(name="wcol", bufs=2))
        outp_pool = ctx.enter_context(tc.tile_pool(name="outp", bufs=2))
        ctx_pool = ctx.enter_context(tc.tile_pool(name="ctxp", bufs=2))
        rc_pool = ctx.enter_context(tc.tile_pool(name="rc", bufs=4))
        # PSUM budget (8 banks): sps 2x[128,1024]=4 (per-head scores chunks,
        # also hosts the projection accumulators and the [128,128] transpose
        # tiles via the same tag), cps 2x[128,1024]=4 (per-head PV accums)
        sps_pool = ctx.enter_context(tc.tile_pool(name="spsp", bufs=2, space="PSUM"))
        cps_pool = ctx.enter_context(tc.tile_pool(name="cpsp", bufs=2, space="PSUM"))

        xt = [xt_pool.tile([P, S], F32R, tag="xt", name=f"xt{i}") for i in range(NK)]
        vp = [vp_pool.tile([P, NH * VW], F32R, tag="vp", name=f"vp{i}") for i in range(NS)]

        # ---- phase 0: load X, build X^T; then V = X @ Wv ----
        with (
            tc.tile_pool(name="xload", bufs=3) as xload,
            tc.tile_pool(name="wvrow", bufs=NK) as wv_pool,
        ):
            for st in range(NS):
                xs = xload.tile([P, H], F32, tag="xs")
                nc.sync.dma_start(out=xs, in_=hs[st * P : (st + 1) * P, :])
                for kt in range(NK):
                    tp = sps_pool.tile([P, P], F32, tag="sps", name="tp0")
                    nc.tensor.transpose(tp, xs[:, kt * P : (kt + 1) * P], ident)
                    nc.vector.tensor_copy(xt[kt][:, st * P : (st + 1) * P], tp)

            wv_t = [wv_pool.tile([P, H], F32R, tag="wv", name=f"wvt{i}") for i in range(NK)]
            if "v" not in phases:
                pass
            else:
              for kt in range(NK):
                nc.sync.dma_start(out=wv_t[kt], in_=_r(wv[kt * P : (kt + 1) * P, :]))
              for st in range(NS):
                  v3d = vp[st].rearrange("p (h c) -> p h c", c=VW)
                  nc.vector.tensor_copy(v3d[:, :, HD : HD + 1], ones_col)
                  for hc in range(2):
                      vps = sps_pool.tile([P, 512], F32, tag="sps", name="vps")
                      for kt in range(NK):
                          nc.tensor.matmul(
                              vps,
                              lhsT=xt[kt][:, st * P : (st + 1) * P],
                              rhs=wv_t[kt][:, hc * 512 : (hc + 1) * 512],
                              start=(kt == 0),
                              stop=(kt == NK - 1),
                          )
                      nc.vector.tensor_add(
                          v3d[:, hc * 8 : (hc + 1) * 8, 0:HD],
                          vps.rearrange("p (h c) -> p h c", c=HD),
                          bv_b.rearrange("p (h c) -> p h c", c=HD)[
                              :, hc * 8 : (hc + 1) * 8, :
                          ],
                      )

        # ---- per head pair: project Q^T/K^T, then attention for both heads ----
        with tc.tile_pool(name="expt", bufs=2) as exp_pool:
            for hp in range(NHP if "qk" in phases else 0):
                wqc = wcol_pool.tile([P, NK, P], F32R, tag="wqc")
                nc.sync.dma_start(
                    out=wqc,
                    in_=_r(wq.rearrange("(t p) m -> p t m", p=P)[
                        :, :, hp * P : (hp + 1) * P
                    ]),
                )
                wkc = wcol_pool.tile([P, NK, P], F32R, tag="wkc")
                nc.sync.dma_start(
                    out=wkc,
                    in_=_r(wk.rearrange("(t p) m -> p t m", p=P)[
                        :, :, hp * P : (hp + 1) * P
                    ]),
                )

                qt_t = qk_pool.tile([P, S], F32R, tag="qt")
                for sc in range(2):
                    qps = sps_pool.tile([P, 512], F32, tag="sps", name="qps")
                    for kt in range(NK):
                        nc.tensor.matmul(
                            qps,
                            lhsT=wqc[:, kt, :],
                            rhs=xt[kt][:, sc * 512 : (sc + 1) * 512],
                            start=(kt == 0),
                            stop=(kt == NK - 1),
                        )
                    nc.vector.tensor_scalar_add(
                        qt_t[:, sc * 512 : (sc + 1) * 512], qps, bq_t[:, hp : hp + 1]
                    )

                kt_t = qk_pool.tile([P, S], F32R, tag="kt")
                for sc in range(2):
                    kps = sps_pool.tile([P, 512], F32, tag="sps", name="kps")
                    for kt in range(NK):
                        nc.tensor.matmul(
                            kps,
                            lhsT=wkc[:, kt, :],
                            rhs=xt[kt][:, sc * 512 : (sc + 1) * 512],
                            start=(kt == 0),
                            stop=(kt == NK - 1),
                        )
                    nc.vector.tensor_scalar_add(
                        kt_t[:, sc * 512 : (sc + 1) * 512], kps, bk_t[:, hp : hp + 1]
                    )

                outp = outp_pool.tile([P, S], F32, tag="outp")
                a_on = "attn" in phases
                exp2 = [
                    exp_pool.tile([P, NK * S], F32R, tag="expt", name=f"exp{i}")
                    for i in range(2)
                ]
                cps2 = [
                    cps_pool.tile([P, 1024], F32, tag="cps", name=f"cps{i}")
                    for i in range(2)
                ]
                for kt in range(NK if a_on else 0):
                    sp2 = [
                        sps_pool.tile([P, S], F32, tag="sps", name=f"sp{i}")
                        for i in range(2)
                    ]
                    for qn in range(2):
                        for sub in range(2):
                            r0 = HD * sub
                            nc.tensor.matmul(
                                sp2[sub][:, qn * 512 : (qn + 1) * 512],
                                lhsT=kt_t[r0 : r0 + HD, kt * P : (kt + 1) * P],
                                rhs=qt_t[r0 : r0 + HD, qn * 512 : (qn + 1) * 512],
                                start=True,
                                stop=True,
                            )
                    for sub in range(2):
                        nc.scalar.activation(
                            exp2[sub][:, kt * S : (kt + 1) * S],
                            sp2[sub],
                            Exp,
                            bias=mask_t[:, kt : kt + 1],
                            scale=1.0 / np.sqrt(HD),
                        )
                    for sub in range(2):
                        h = 2 * hp + sub
                        for qn in range(2):
                            nc.tensor.matmul(
                                cps2[sub][:VW, qn * 512 : (qn + 1) * 512],
                                lhsT=vp[kt][:, h * VW : (h + 1) * VW],
                                rhs=exp2[sub][
                                    :, kt * S + qn * 512 : kt * S + qn * 512 + 512
                                ],
                                start=(kt == 0),
                                stop=(kt == NK - 1),
                            )
                for sub in range(2 if a_on else 0):
                    r0 = HD * sub
                    ctx_sb = ctx_pool.tile([VW, S], F32, tag="ctx")
                    nc.vector.tensor_copy(ctx_sb, cps2[sub][:VW, :])
                    for qt in range(NS):
                        tp = sps_pool.tile([P, P], F32, tag="sps", name="tpc")
                        nc.tensor.transpose(
                            tp[:, :VW],
                            ctx_sb[:, qt * P : (qt + 1) * P],
                            ident[:VW, :VW],
                        )
                        rc = rc_pool.tile([P, 1], F32, tag="rc")
                        nc.vector.reciprocal(rc, tp[:, HD : HD + 1])
                        nc.vector.tensor_scalar_mul(
                            outp[:, qt * P + r0 : qt * P + r0 + HD],
                            tp[:, 0:HD],
                            rc,
                        )
                if "attn" in phases:
                    nc.sync.dma_start(
                        out=out.rearrange("(q p) c -> p q c", p=P)[
                            :, :, hp * P : (hp + 1) * P
                        ],
                        in_=outp.rearrange("p (q c) -> p q c", c=P),
                    )


def _split_excess_waits(nc):
    """This walrus build rejects instructions with more than a couple of sem
    waits. Hoist excess waits onto injected same-engine NoOps that execute
    immediately before the overfull instruction (program order per engine is
    the basic-block order, so the waits still complete first)."""
    counter = 0
    for func in nc.m.functions:
        for block in func.blocks:
            insts = block.instructions
            out = []
            changed = False
            for inst in insts:
                si = inst.sync_info
                limit = 2 if type(inst).__name__ == "InstEventSemaphore" else 1
                if si is not None and len(si.on_wait) > limit:
                    waits = list(si.on_wait)
                    for w in waits[limit:]:
                        nop = mybir.InstNoOp(
                            name=f"I-wsplit-{counter}", engine=inst.engine
                        )
                        counter += 1
                        nop.sync_info = mybir.SyncInfo(on_wait=[w], on_update=[])
                        nop.debug = inst.debug
                        out.append(nop)
                    si.on_wait = waits[:limit]
                    inst.sync_info = si
                    changed = True
                out.append(inst)
            if changed:
                block.instructions = out


_NC_CACHE = {}


def _build(split_waits=True, n_reps=1, phases=("x", "v", "qk", "attn"), **kw):
    global _NC_CACHE
    key = (n_reps, tuple(phases), tuple(sorted(kw.items())))
    if split_waits and key in _NC_CACHE:
        return _NC_CACHE[key]
    _patch_tile_drain()
    nc = bass.Bass(target_bir_lowering=False, debug=False)
    hs = nc.dram_tensor("hs", [S, H], F32, kind="ExternalInput").ap()
    mask = nc.dram_tensor("mask", [S], F32, kind="ExternalInput").ap()
    wq = nc.dram_tensor("wq", [H, H], F32, kind="ExternalInput").ap()
    bq = nc.dram_tensor("bq", [H], F32, kind="ExternalInput").ap()
    wk = nc.dram_tensor("wk", [H, H], F32, kind="ExternalInput").ap()
    bk = nc.dram_tensor("bk", [H], F32, kind="ExternalInput").ap()
    wv = nc.dram_tensor("wv", [H, H], F32, kind="ExternalInput").ap()
    bv = nc.dram_tensor("bv", [H], F32, kind="ExternalInput").ap()
    out = nc.dram_tensor("out", [S, H], F32, kind="ExternalOutput").ap()
    from contextlib import ExitStack

    with tile.TileContext(nc) as tc, ExitStack() as cctx:
        consts = build_consts(nc, tc, cctx, mask, bq, bk, bv)
        for _ in range(n_reps):
            build_kernel(
                nc, tc, consts, hs, mask, wq, bq, wk, bk, wv, bv, out,
                phases=phases, **kw
            )
    if not split_waits:
        return nc
    _split_excess_waits(nc)
    _NC_CACHE[key] = nc
    return nc


def make_in_maps(hidden_states, attention_mask, Wq, bq, Wk, bk, Wv, bv):
    f = np.ascontiguousarray
    maps = []
    for i in range(N_CORES):
        maps.append(
            {
                "hs": f(np.asarray(hidden_states[i], dtype=np.float32)),
                "mask": f(
                    np.asarray(attention_mask[i], dtype=np.float32).reshape(S)
                ),
                "wq": f(np.asarray(Wq, dtype=np.float32)),
                "bq": f(np.asarray(bq, dtype=np.float32)),
                "wk": f(np.asarray(Wk, dtype=np.float32)),
                "bk": f(np.asarray(bk, dtype=np.float32)),
                "wv": f(np.asarray(Wv, dtype=np.float32)),
                "bv": f(np.asarray(bv, dtype=np.float32)),
            }
        )
    return maps


def run(in_maps, **kwargs):
    nc = _build()
    return run_bass_kernel_spmd(nc, in_maps, core_ids=list(range(N_CORES)), **kwargs)


def kernel(hidden_states, attention_mask, Wq, bq, Wk, bk, Wv, bv):
    in_maps = make_in_maps(hidden_states, attention_mask, Wq, bq, Wk, bk, Wv, bv)
    res = run(in_maps)
    return np.stack([res.results[i]["out"] for i in range(N_CORES)], axis=0)



# revision 2
# speedup vs baseline: 1.4543x; 1.4543x over previous
"""BERT self-attention (B=8, S=1024, H=1024, 16 heads) on 8 TRN2 NeuronCores.

Sharding: data-parallel over batch - core i computes batch element i.

Per-core dataflow, software-pipelined across heads AND reps:
  X^T        via PE transposes (f32r, 1.5 c/row)
  V          = X @ Wv -> fp8e4 tiles [key, head, d | ones] for DoubleRow PV
  Q^T, K^T   = Wq/Wk col-blocks vs X^T (f32r), per head-pair, prefetched
  scores^T   = K_h @ Q_h^T per head ([key, q], contraction 64, f32r)
  P          = exp(scores/8 + mask - 3) on ScalarE, written as fp8e4
               (softmax is shift-invariant; -3 keeps exp under fp8e4 max 448)
  ctx'^T     = V' @ P via fp8 DoubleRow matmuls (256-key contraction, 0.5 c/row)
               rows 0..63 = unnormalized ctx^T, row 64 = denominator
  normalize  before transpose: recip (DVE) + partition_broadcast + mul (Pool)
  ctx        = PE transposes (f32r) -> outp -> DMA

Emission interleaves a "filler" stream (next pair's projections, next rep's
X^T/V/weight loads) into the attention windows so the PE and ACT engines both
stay busy; per-engine program order is the schedule.
"""

import itertools
import numpy as np

import concourse.bass as bass
import concourse.mybir as mybir
import concourse.tile as tile
from concourse.bass_utils import run_bass_kernel_spmd
from concourse.masks import make_identity
from concourse.vector_clock import ScopedClock

S = 1024
H = 1024
NH = 16
HD = 64
P = 128
NS = S // P
NK = H // P
NHP = NH // 2
VW = HD + 1
F32 = mybir.dt.float32
F32R = mybir.dt.float32r
BF16 = mybir.dt.bfloat16
Exp = mybir.ActivationFunctionType.Exp
N_CORES = 8

EXP_SHIFT = -3.0  # exp(s + mask - 3): keeps P small; softmax is shift-invariant

_DRAIN_WAIT_CHUNK = 1
_patched = False


def _patch_tile_drain():
    """The walrus build in this container rejects instructions carrying more
    than a few sem waits; TileContext's tail drain waits on every live sem in
    one instruction. Split it into one drain per wait."""
    global _patched
    if _patched:
        return
    _patched = True

    def _drain_and_barrier(self, tick_clock, wait_clock):
        drain_inst = self.nc.sync.drain()
        wait_clock.add_sem_waits(
            drain_inst.ins, ScopedClock({None: tick_clock.global_clock})
        )
        si = drain_inst.ins.sync_info
        if si is not None and len(si.on_wait) > _DRAIN_WAIT_CHUNK:
            waits = list(si.on_wait)
            si.on_wait = waits[:_DRAIN_WAIT_CHUNK]
            drain_inst.ins.sync_info = si
            for i in range(_DRAIN_WAIT_CHUNK, len(waits), _DRAIN_WAIT_CHUNK):
                extra = self.nc.sync.drain()
                extra.ins.sync_info = mybir.SyncInfo(
                    on_wait=waits[i : i + _DRAIN_WAIT_CHUNK], on_update=[]
                )
        self.nc.all_engine_barrier()
        popped = self.nc._tile_sem_poison_stack.pop()
        assert popped is self._sem_poison
        self.nc.clear_and_free_semaphores(list(self.sems.allocated().values()))
        self.nc.all_engine_barrier()

    tile.TileContext._drain_and_barrier = _drain_and_barrier


def _r(ap):
    return ap.bitcast(F32R)


def build_consts(nc, tc, ctx, mask, bq, bk, bv):
    singles = ctx.enter_context(tc.tile_pool(name="singles", bufs=1))
    ident = singles.tile([P, P], F32)
    make_identity(nc, ident)
    identr = singles.tile([P, P], F32R)
    nc.vector.tensor_copy(identr, ident)  # rounded f32r copy for transposes
    mask_t = singles.tile([P, NK], F32)
    nc.sync.dma_start(out=mask_t, in_=mask.rearrange("(t p) -> p t", p=P))
    mask3_t = singles.tile([P, NK], F32)
    nc.vector.tensor_scalar_add(mask3_t, mask_t, EXP_SHIFT)
    bq_t = singles.tile([P, NHP], F32)
    nc.sync.dma_start(out=bq_t, in_=bq.rearrange("(t p) -> p t", p=P))
    bk_t = singles.tile([P, NHP], F32)
    nc.sync.dma_start(out=bk_t, in_=bk.rearrange("(t p) -> p t", p=P))
    bv_b = singles.tile([P, H], F32)
    nc.gpsimd.dma_start(
        out=bv_b,
        in_=bass.AP(tensor=bv.tensor, offset=bv.offset, ap=[[0, P], bv.ap[0]]),
    )
    return ident, identr, mask3_t, bq_t, bk_t, bv_b


class Pools:
    """Long-lived pools shared across reps (rings provide cross-rep overlap)."""

    def __init__(self, tc, ctx):
        ec = ctx.enter_context
        self.xt = ec(tc.tile_pool(name="xt", bufs=NK + 2))
        self.wv = ec(tc.tile_pool(name="wv", bufs=NK))
        self.xs = ec(tc.tile_pool(name="xs", bufs=3))
        self.vp = ec(tc.tile_pool(name="vp", bufs=2))
        self.qk = ec(tc.tile_pool(name="qk", bufs=4))
        self.wcol = ec(tc.tile_pool(name="wcol", bufs=2))
        self.expp = ec(tc.tile_pool(name="expp", bufs=3))
        self.csb = ec(tc.tile_pool(name="csb", bufs=2))
        self.outp = ec(tc.tile_pool(name="outp", bufs=2))
        # PSUM: sps 2x[128,1024]=4 banks, cps 1x[128,1024]=2, pps 2x[128,512]=2
        self.sps = ec(tc.tile_pool(name="spsp", bufs=2, space="PSUM"))
        self.cps = ec(tc.tile_pool(name="cpsp", bufs=1, space="PSUM"))
        self.pps = ec(tc.tile_pool(name="ppsp", bufs=2, space="PSUM"))


class RepState:
    def __init__(self):
        self.xt = None  # list of [P, S] f32r X^T tiles
        self.wv_t = None
        self.vp4 = None  # [P, NS, NH, VW] fp8 V' tile
        self.proj_tiles = {}  # pair -> (qt_t, kt_t)


def emit_phase_x_steps(nc, po, consts, hs, wv, st_range, rs):
    """Load X tiles, build X^T via PE transposes, project V into fp8 vp4.
    Yields at matmul-pair granularity for interleaving."""
    ident, identr, mask3_t, bq_t, bk_t, bv_b = consts
    for st in st_range:
        xs = po.xs.tile([P, H], F32R, tag="xs", name="xs")
        nc.sync.dma_start(out=xs, in_=_r(hs[st * P : (st + 1) * P, :]))
        for kt in range(NK):
            tp = po.pps.tile([P, 512], F32, tag="pps", name="tpx")
            nc.tensor.transpose(
                _r(tp[:, :P]), xs[:, kt * P : (kt + 1) * P], identr
            )
            nc.vector.tensor_copy(rs.xt[kt][:, st * P : (st + 1) * P], tp[:, :P])
            if kt % 2 == 1:
                yield
        # V projection for this s-tile (keys st*128..) into vp4
        v3 = rs.vp4
        bv3 = bv_b.rearrange("p (h c) -> p h c", c=HD)
        for hc in range(2):
            vps = po.pps.tile([P, 512], F32, tag="pps", name="vps")
            for kt in range(NK):
                nc.tensor.matmul(
                    vps,
                    lhsT=rs.xt[kt][:, st * P : (st + 1) * P],
                    rhs=rs.wv_t[kt][:, hc * 512 : (hc + 1) * 512],
                    start=(kt == 0),
                    stop=(kt == NK - 1),
                )
                if kt % 2 == 1:
                    yield
            nc.vector.tensor_add(
                v3[:, st, hc * 8 : (hc + 1) * 8, 0:HD],
                vps.rearrange("p (h c) -> p h c", c=HD),
                bv3[:, hc * 8 : (hc + 1) * 8, :],
            )
            yield


def emit_rep_setup(nc, po, rs):
    """Allocate per-rep tiles that phase X fills."""
    rs.xt = [po.xt.tile([P, S], F32R, tag="xt", name=f"xt{i}") for i in range(NK)]
    rs.vp4 = po.vp.tile([P, NS, NH, VW], BF16, tag="vp4", name="vp4")
    nc.gpsimd.memset(rs.vp4[:, :, :, HD : HD + 1], 1.0)


def emit_wv_loads(nc, po, rs, wv):
    rs.wv_t = [
        po.wv.tile([P, H], F32R, tag="wv", name=f"wvt{i}") for i in range(NK)
    ]
    for kt in range(NK):
        nc.sync.dma_start(out=rs.wv_t[kt], in_=_r(wv[kt * P : (kt + 1) * P, :]))


def emit_proj_steps(nc, po, consts, wq, wk, rs, hp):
    """Q^T/K^T projection for head pair hp. Yields for interleaving."""
    ident, identr, mask3_t, bq_t, bk_t, bv_b = consts
    wqc = po.wcol.tile([P, NK, P], F32R, tag="wqc", name="wqc")
    nc.sync.dma_start(
        out=wqc,
        in_=_r(wq.rearrange("(t p) m -> p t m", p=P)[:, :, hp * P : (hp + 1) * P]),
    )
    wkc = po.wcol.tile([P, NK, P], F32R, tag="wkc", name="wkc")
    nc.sync.dma_start(
        out=wkc,
        in_=_r(wk.rearrange("(t p) m -> p t m", p=P)[:, :, hp * P : (hp + 1) * P]),
    )
    qt_t = po.qk.tile([P, S], F32R, tag="qt", name="qt")
    kt_t = po.qk.tile([P, S], F32R, tag="kt", name="ktt")
    rs.proj_tiles[hp] = (qt_t, kt_t)
    for wc, bias_t, dst in ((wqc, bq_t, qt_t), (wkc, bk_t, kt_t)):
        for sc in range(2):
            pp = po.pps.tile([P, 512], F32, tag="pps", name="pp")
            for kt in range(NK):
                nc.tensor.matmul(
                    pp,
                    lhsT=wc[:, kt, :],
                    rhs=rs.xt[kt][:, sc * 512 : (sc + 1) * 512],
                    start=(kt == 0),
                    stop=(kt == NK - 1),
                )
                if kt % 2 == 1:
                    yield
            nc.vector.tensor_scalar_add(
                dst[:, sc * 512 : (sc + 1) * 512], pp, bias_t[:, hp : hp + 1]
            )
            yield


def build_all(nc, tc, consts, hs, wq, wk, wv, out, n_reps, interleave=True, defer_tail=True):
    from contextlib import ExitStack

    ident, identr, mask3_t, bq_t, bk_t, bv_b = consts

    ctx = ExitStack()
    with ctx:
        po = Pools(tc, ctx)

        reps = [RepState() for _ in range(n_reps)]

        # Prologue: rep 0 phase X + pair-0 projections, not overlapped.
        emit_wv_loads(nc, po, reps[0], wv)
        emit_rep_setup(nc, po, reps[0])
        for _ in emit_phase_x_steps(nc, po, consts, hs, wv, range(NS), reps[0]):
            pass
        for _ in emit_proj_steps(nc, po, consts, wq, wk, reps[0], 0):
            pass

        def rep_filler(r):
            """Work interleaved into rep r's attention: rep r pairs 1..7
            projections, then rep r+1's phase X and pair-0 projections."""
            for p2 in range(1, NHP):
                yield from emit_proj_steps(nc, po, consts, wq, wk, reps[r], p2)
            if r + 1 < n_reps:
                emit_wv_loads(nc, po, reps[r + 1], wv)
                emit_rep_setup(nc, po, reps[r + 1])
                yield from emit_phase_x_steps(
                    nc, po, consts, hs, wv, range(NS), reps[r + 1]
                )
                yield from emit_proj_steps(nc, po, consts, wq, wk, reps[r + 1], 0)

        for r in range(n_reps):
            filler = rep_filler(r)

            def adv(n):
                if not interleave and n < 10**8:
                    return
                for _ in itertools.islice(filler, n):
                    pass

            rs = reps[r]
            pending_tail = None
            outp_cur = None
            for h in range(NH):
                hp, sub = divmod(h, 2)
                r0 = sub * HD
                qt_t, kt_t = rs.proj_tiles[hp]
                if sub == 0:
                    outp_cur = po.outp.tile([P, S], F32, tag="outp", name="outp")
                cps = po.cps.tile([P, S], F32, tag="cps", name="cps")
                ex = None
                for kt in range(NK):
                    if kt % 2 == 0:
                        ex = po.expp.tile([P, 2, S], BF16, tag="expp", name="expt")
                    sp = po.sps.tile([P, S], F32, tag="sps", name="sp")
                    for qn in range(2):
                        nc.tensor.matmul(
                            sp[:, qn * 512 : (qn + 1) * 512],
                            lhsT=kt_t[r0 : r0 + HD, kt * P : (kt + 1) * P],
                            rhs=qt_t[r0 : r0 + HD, qn * 512 : (qn + 1) * 512],
                            start=True,
                            stop=True,
                        )
                    nc.scalar.activation(
                        ex[:, kt % 2, :],
                        sp,
                        Exp,
                        bias=mask3_t[:, kt : kt + 1],
                        scale=1.0 / np.sqrt(HD),
                    )
                    if kt == 2 and pending_tail is not None:
                        pending_tail()
                        pending_tail = None
                    adv(2)
                    lhsT = rs.vp4[:, kt : kt + 1, h : h + 1, :]
                    for qn in range(2):
                        nc.tensor.matmul(
                            cps[0:VW, qn * 512 : (qn + 1) * 512],
                            lhsT=lhsT,
                            rhs=ex[:, kt % 2 : kt % 2 + 1, qn * 512 : (qn + 1) * 512],
                            start=(kt == 0),
                            stop=(kt == NK - 1),
                        )
                # Tail: drain cps (DVE copy, frees the single cps buffer fast);
                # transpose unnormalized ctx'^T blocks [65,128]->[128,65] so the
                # denominator becomes per-partition, then reciprocal + per-qtile
                # tensor_scalar_mul normalize during the outp drain. PE parts
                # are deferred into the next head's window.
                csb = po.csb.tile([P, S], F32, tag="csb", name="csb")
                nc.vector.tensor_copy(csb[0:VW, :], cps[0:VW, :])

                def make_tail(csb=csb, outp=outp_cur, r0=r0, hp=hp, last=(sub == 1)):
                    def tail():
                        outp3 = outp.rearrange("p (q c) -> p q c", c=P)
                        rcd = po.csb.tile([P, NS], F32, tag="rcd", name="rcd")
                        rcd3 = rcd.rearrange("p (a b) -> p a b", b=1)
                        for half in range(2):
                            tp = po.pps.tile([P, 512], F32, tag="pps", name="tpc")
                            for j in range(4):
                                qi = half * 4 + j
                                nc.tensor.transpose(
                                    tp[:, j * VW : (j + 1) * VW],
                                    csb[0:VW, qi * P : (qi + 1) * P],
                                    ident[0:VW, 0:VW],
                                )
                            tp3 = tp[:, 0 : 4 * VW].rearrange("p (a b) -> p a b", b=VW)
                            nc.vector.reciprocal(
                                rcd3[:, half * 4 : (half + 1) * 4, :],
                                tp3[:, 0:4, HD : HD + 1],
                            )
                            for j in range(4):
                                qi = half * 4 + j
                                nc.vector.tensor_scalar_mul(
                                    outp3[:, qi, r0 : r0 + HD],
                                    tp[:, j * VW : j * VW + HD],
                                    rcd[:, qi : qi + 1],
                                )
                        if last:
                            nc.gpsimd.dma_start(
                                out=out.rearrange("(q p) c -> p q c", p=P)[
                                    :, :, hp * P : (hp + 1) * P
                                ],
                                in_=outp3,
                            )

                    return tail

                if defer_tail:
                    pending_tail = make_tail()
                else:
                    make_tail()()
            if pending_tail is not None:
                pending_tail()
            pending_tail = None
            # drain any filler left (end of last rep)
            adv(10**9)


def _split_excess_waits(nc):
    """This walrus build rejects instructions with more than a couple of sem
    waits. Hoist excess waits onto injected same-engine NoOps that execute
    immediately before the overfull instruction."""
    counter = 0
    for func in nc.m.functions:
        for block in func.blocks:
            insts = block.instructions
            out = []
            changed = False
            for inst in insts:
                si = inst.sync_info
                limit = 2 if type(inst).__name__ == "InstEventSemaphore" else 1
                if si is not None and len(si.on_wait) > limit:
                    waits = list(si.on_wait)
                    for w in waits[limit:]:
                        nop = mybir.InstNoOp(
                            name=f"I-wsplit-{counter}", engine=inst.engine
                        )
                        counter += 1
                        nop.sync_info = mybir.SyncInfo(on_wait=[w], on_update=[])
                        nop.debug = inst.debug
                        out.append(nop)
                    si.on_wait = waits[:limit]
                    inst.sync_info = si
                    changed = True
                out.append(inst)
            if changed:
                block.instructions = out


_NC_CACHE = {}


def _build(split_waits=True, n_reps=1, **kw):
    global _NC_CACHE
    key = (n_reps, tuple(sorted(kw.items())))
    if split_waits and key in _NC_CACHE:
        return _NC_CACHE[key]
    _patch_tile_drain()
    nc = bass.Bass(target_bir_lowering=False, debug=False)
    hs = nc.dram_tensor("hs", [S, H], F32, kind="ExternalInput").ap()
    mask = nc.dram_tensor("mask", [S], F32, kind="ExternalInput").ap()
    wq = nc.dram_tensor("wq", [H, H], F32, kind="ExternalInput").ap()
    bq = nc.dram_tensor("bq", [H], F32, kind="ExternalInput").ap()
    wk = nc.dram_tensor("wk", [H, H], F32, kind="ExternalInput").ap()
    bk = nc.dram_tensor("bk", [H], F32, kind="ExternalInput").ap()
    wv = nc.dram_tensor("wv", [H, H], F32, kind="ExternalInput").ap()
    bv = nc.dram_tensor("bv", [H], F32, kind="ExternalInput").ap()
    out = nc.dram_tensor("out", [S, H], F32, kind="ExternalOutput").ap()
    from contextlib import ExitStack

    with tile.TileContext(nc) as tc, ExitStack() as cctx:
        consts = build_consts(nc, tc, cctx, mask, bq, bk, bv)
        build_all(nc, tc, consts, hs, wq, wk, wv, out, n_reps, **kw)
    if not split_waits:
        return nc
    _split_excess_waits(nc)
    _NC_CACHE[key] = nc
    return nc


def make_in_maps(hidden_states, attention_mask, Wq, bq, Wk, bk, Wv, bv):
    f = np.ascontiguousarray
    maps = []
    for i in range(N_CORES):
        maps.append(
            {
                "hs": f(np.asarray(hidden_states[i], dtype=np.float32)),
                "mask": f(np.asarray(attention_mask[i], dtype=np.float32).reshape(S)),
                "wq": f(np.asarray(Wq, dtype=np.float32)),
                "bq": f(np.asarray(bq, dtype=np.float32)),
                "wk": f(np.asarray(Wk, dtype=np.float32)),
                "bk": f(np.asarray(bk, dtype=np.float32)),
                "wv": f(np.asarray(Wv, dtype=np.float32)),
                "bv": f(np.asarray(bv, dtype=np.float32)),
            }
        )
    return maps


def run(in_maps, **kwargs):
    nc = _build()
    return run_bass_kernel_spmd(nc, in_maps, core_ids=list(range(N_CORES)), **kwargs)


def kernel(hidden_states, attention_mask, Wq, bq, Wk, bk, Wv, bv):
    in_maps = make_in_maps(hidden_states, attention_mask, Wq, bq, Wk, bk, Wv, bv)
    res = run(in_maps)
    return np.stack([res.results[i]["out"] for i in range(N_CORES)], axis=0)


# revision 3
# speedup vs baseline: 1.4770x; 1.0156x over previous
"""BERT self-attention (B=8, S=1024, H=1024, 16 heads) on 8 TRN2 NeuronCores.

Sharding: data-parallel over batch - core i computes batch element i.

Per-core dataflow, software-pipelined across heads AND reps:
  X^T        via PE transposes (f32r, 1.5 c/row)
  V          = X @ Wv -> bf16 tiles [key, head, d | ones-column]
  Q^T, K^T   = Wq/Wk col-blocks vs X^T (f32r), per head-pair, prefetched
  scores^T   = K_h @ Q_h^T per head ([key, q] layout, contraction 64, f32r)
  P          = exp(scores/8 + mask - 3) on ScalarE, written as bf16
               (softmax is shift-invariant so the -3 cancels exactly)
  ctx'^T     = V' @ P (bf16): rows 0..63 = unnormalized ctx^T, row 64 = denom
  ctx        = PE transposes of [65,128] blocks -> denominator becomes
               per-partition -> reciprocal + tensor_scalar_mul normalize
               during the outp drain -> DMA

Emission interleaves a "filler" stream (next pair's projections, next rep's
X^T/V/weight loads) into the attention windows so the PE and ACT engines both
stay busy; per-engine program order is the schedule.
"""

import itertools
import numpy as np

import concourse.bass as bass
import concourse.mybir as mybir
import concourse.tile as tile
from concourse.bass_utils import run_bass_kernel_spmd
from concourse.masks import make_identity
from concourse.vector_clock import ScopedClock

S = 1024
H = 1024
NH = 16
HD = 64
P = 128
NS = S // P
NK = H // P
NHP = NH // 2
VW = HD + 1
F32 = mybir.dt.float32
F32R = mybir.dt.float32r
BF16 = mybir.dt.bfloat16
Exp = mybir.ActivationFunctionType.Exp
N_CORES = 8

EXP_SHIFT = -3.0  # exp(s + mask - 3): keeps P small; softmax is shift-invariant

_DRAIN_WAIT_CHUNK = 1
_patched = False


def _patch_tile_drain():
    """The walrus build in this container rejects instructions carrying more
    than a few sem waits; TileContext's tail drain waits on every live sem in
    one instruction. Split it into one drain per wait."""
    global _patched
    if _patched:
        return
    _patched = True

    def _drain_and_barrier(self, tick_clock, wait_clock):
        drain_inst = self.nc.sync.drain()
        wait_clock.add_sem_waits(
            drain_inst.ins, ScopedClock({None: tick_clock.global_clock})
        )
        si = drain_inst.ins.sync_info
        if si is not None and len(si.on_wait) > _DRAIN_WAIT_CHUNK:
            waits = list(si.on_wait)
            si.on_wait = waits[:_DRAIN_WAIT_CHUNK]
            drain_inst.ins.sync_info = si
            for i in range(_DRAIN_WAIT_CHUNK, len(waits), _DRAIN_WAIT_CHUNK):
                extra = self.nc.sync.drain()
                extra.ins.sync_info = mybir.SyncInfo(
                    on_wait=waits[i : i + _DRAIN_WAIT_CHUNK], on_update=[]
                )
        self.nc.all_engine_barrier()
        popped = self.nc._tile_sem_poison_stack.pop()
        assert popped is self._sem_poison
        self.nc.clear_and_free_semaphores(list(self.sems.allocated().values()))
        self.nc.all_engine_barrier()

    tile.TileContext._drain_and_barrier = _drain_and_barrier


def _r(ap):
    return ap.bitcast(F32R)


def build_consts(nc, tc, ctx, mask, bq, bk, bv):
    singles = ctx.enter_context(tc.tile_pool(name="singles", bufs=1))
    ident = singles.tile([P, P], F32)
    make_identity(nc, ident)
    identr = singles.tile([P, P], F32R)
    nc.vector.tensor_copy(identr, ident)  # rounded f32r copy for transposes
    mask_t = singles.tile([P, NK], F32)
    nc.sync.dma_start(out=mask_t, in_=mask.rearrange("(t p) -> p t", p=P))
    mask3_t = singles.tile([P, NK], F32)
    nc.vector.tensor_scalar_add(mask3_t, mask_t, EXP_SHIFT)
    bq_t = singles.tile([P, NHP], F32)
    nc.sync.dma_start(out=bq_t, in_=bq.rearrange("(t p) -> p t", p=P))
    bk_t = singles.tile([P, NHP], F32)
    nc.sync.dma_start(out=bk_t, in_=bk.rearrange("(t p) -> p t", p=P))
    bv_b = singles.tile([P, H], F32)
    nc.gpsimd.dma_start(
        out=bv_b,
        in_=bass.AP(tensor=bv.tensor, offset=bv.offset, ap=[[0, P], bv.ap[0]]),
    )
    return ident, identr, mask3_t, bq_t, bk_t, bv_b


class Pools:
    """Long-lived pools shared across reps (rings provide cross-rep overlap)."""

    def __init__(self, tc, ctx):
        ec = ctx.enter_context
        self.xt = ec(tc.tile_pool(name="xt", bufs=NK + 2))
        self.wv = ec(tc.tile_pool(name="wv", bufs=NK))
        self.xs = ec(tc.tile_pool(name="xs", bufs=3))
        self.vp = ec(tc.tile_pool(name="vp", bufs=2))
        self.qk = ec(tc.tile_pool(name="qk", bufs=4))
        self.wcol = ec(tc.tile_pool(name="wcol", bufs=2))
        self.expp = ec(tc.tile_pool(name="expp", bufs=3))
        self.csb = ec(tc.tile_pool(name="csb", bufs=2))
        self.outp = ec(tc.tile_pool(name="outp", bufs=2))
        # PSUM: sps 2x[128,1024]=4 banks, cps 1x[128,1024]=2, pps 2x[128,512]=2
        self.sps = ec(tc.tile_pool(name="spsp", bufs=2, space="PSUM"))
        self.cps = ec(tc.tile_pool(name="cpsp", bufs=1, space="PSUM"))
        self.pps = ec(tc.tile_pool(name="ppsp", bufs=2, space="PSUM"))


class RepState:
    def __init__(self):
        self.xt = None  # list of [P, S] f32r X^T tiles
        self.wv_t = None
        self.vp4 = None  # [P, NS, NH, VW] fp8 V' tile
        self.proj_tiles = {}  # pair -> (qt_t, kt_t)


def emit_phase_x_steps(nc, po, consts, hs, wv, st_range, rs):
    """Load X tiles, build X^T via PE transposes, project V into fp8 vp4.
    Yields at matmul-pair granularity for interleaving."""
    ident, identr, mask3_t, bq_t, bk_t, bv_b = consts
    for st in st_range:
        xs = po.xs.tile([P, H], F32R, tag="xs", name="xs")
        nc.sync.dma_start(out=xs, in_=_r(hs[st * P : (st + 1) * P, :]))
        for kt in range(NK):
            tp = po.pps.tile([P, 512], F32, tag="pps", name="tpx")
            nc.tensor.transpose(
                _r(tp[:, :P]), xs[:, kt * P : (kt + 1) * P], identr
            )
            nc.vector.tensor_copy(rs.xt[kt][:, st * P : (st + 1) * P], tp[:, :P])
            if kt % 2 == 1:
                yield
        # V projection for this s-tile (keys st*128..) into vp4
        v3 = rs.vp4
        bv3 = bv_b.rearrange("p (h c) -> p h c", c=HD)
        for hc in range(2):
            vps = po.pps.tile([P, 512], F32, tag="pps", name="vps")
            for kt in range(NK):
                nc.tensor.matmul(
                    vps,
                    lhsT=rs.xt[kt][:, st * P : (st + 1) * P],
                    rhs=rs.wv_t[kt][:, hc * 512 : (hc + 1) * 512],
                    start=(kt == 0),
                    stop=(kt == NK - 1),
                )
                if kt % 2 == 1:
                    yield
            nc.vector.tensor_add(
                v3[:, st, hc * 8 : (hc + 1) * 8, 0:HD],
                vps.rearrange("p (h c) -> p h c", c=HD),
                bv3[:, hc * 8 : (hc + 1) * 8, :],
            )
            yield


def emit_rep_setup(nc, po, rs):
    """Allocate per-rep tiles that phase X fills."""
    rs.xt = [po.xt.tile([P, S], F32R, tag="xt", name=f"xt{i}") for i in range(NK)]
    rs.vp4 = po.vp.tile([P, NS, NH, VW], BF16, tag="vp4", name="vp4")
    nc.gpsimd.memset(rs.vp4[:, :, :, HD : HD + 1], 1.0)


def emit_wv_loads(nc, po, rs, wv):
    rs.wv_t = [
        po.wv.tile([P, H], F32R, tag="wv", name=f"wvt{i}") for i in range(NK)
    ]
    for kt in range(NK):
        nc.sync.dma_start(out=rs.wv_t[kt], in_=_r(wv[kt * P : (kt + 1) * P, :]))


def emit_proj_steps(nc, po, consts, wq, wk, rs, hp):
    """Q^T/K^T projection for head pair hp. Yields for interleaving."""
    ident, identr, mask3_t, bq_t, bk_t, bv_b = consts
    wqc = po.wcol.tile([P, NK, P], F32R, tag="wqc", name="wqc")
    nc.sync.dma_start(
        out=wqc,
        in_=_r(wq.rearrange("(t p) m -> p t m", p=P)[:, :, hp * P : (hp + 1) * P]),
    )
    wkc = po.wcol.tile([P, NK, P], F32R, tag="wkc", name="wkc")
    nc.sync.dma_start(
        out=wkc,
        in_=_r(wk.rearrange("(t p) m -> p t m", p=P)[:, :, hp * P : (hp + 1) * P]),
    )
    qt_t = po.qk.tile([P, S], F32R, tag="qt", name="qt")
    kt_t = po.qk.tile([P, S], F32R, tag="kt", name="ktt")
    rs.proj_tiles[hp] = (qt_t, kt_t)
    for wc, bias_t, dst in ((wqc, bq_t, qt_t), (wkc, bk_t, kt_t)):
        for sc in range(2):
            pp = po.pps.tile([P, 512], F32, tag="pps", name="pp")
            for kt in range(NK):
                nc.tensor.matmul(
                    pp,
                    lhsT=wc[:, kt, :],
                    rhs=rs.xt[kt][:, sc * 512 : (sc + 1) * 512],
                    start=(kt == 0),
                    stop=(kt == NK - 1),
                )
                if kt % 2 == 1:
                    yield
            nc.vector.tensor_scalar_add(
                dst[:, sc * 512 : (sc + 1) * 512], pp, bias_t[:, hp : hp + 1]
            )
            yield


def build_all(nc, tc, consts, hs, wq, wk, wv, out, n_reps, interleave=True, defer_tail=True):
    from contextlib import ExitStack

    ident, identr, mask3_t, bq_t, bk_t, bv_b = consts

    ctx = ExitStack()
    with ctx:
        po = Pools(tc, ctx)

        reps = [RepState() for _ in range(n_reps)]

        # Prologue: rep 0 phase X + pair-0 projections, not overlapped.
        emit_wv_loads(nc, po, reps[0], wv)
        emit_rep_setup(nc, po, reps[0])
        for _ in emit_phase_x_steps(nc, po, consts, hs, wv, range(NS), reps[0]):
            pass
        for _ in emit_proj_steps(nc, po, consts, wq, wk, reps[0], 0):
            pass

        def rep_filler(r):
            """Work interleaved into rep r's attention: rep r pairs 1..7
            projections, then rep r+1's phase X and pair-0 projections."""
            for p2 in range(1, NHP):
                yield from emit_proj_steps(nc, po, consts, wq, wk, reps[r], p2)
            if r + 1 < n_reps:
                emit_wv_loads(nc, po, reps[r + 1], wv)
                emit_rep_setup(nc, po, reps[r + 1])
                yield from emit_phase_x_steps(
                    nc, po, consts, hs, wv, range(NS), reps[r + 1]
                )
                yield from emit_proj_steps(nc, po, consts, wq, wk, reps[r + 1], 0)

        for r in range(n_reps):
            filler = rep_filler(r)

            def adv(n):
                if not interleave and n < 10**8:
                    return
                for _ in itertools.islice(filler, n):
                    pass

            rs = reps[r]
            pending_tail = None
            outp_cur = None
            for h in range(NH):
                hp, sub = divmod(h, 2)
                r0 = sub * HD
                qt_t, kt_t = rs.proj_tiles[hp]
                if sub == 0:
                    outp_cur = po.outp.tile([P, S], F32, tag="outp", name="outp")
                cps = po.cps.tile([P, S], F32, tag="cps", name="cps")
                ex = None
                for kt in range(NK):
                    if kt % 2 == 0:
                        ex = po.expp.tile([P, 2, S], BF16, tag="expp", name="expt")
                    sp = po.sps.tile([P, S], F32, tag="sps", name="sp")
                    for qn in range(2):
                        nc.tensor.matmul(
                            sp[:, qn * 512 : (qn + 1) * 512],
                            lhsT=kt_t[r0 : r0 + HD, kt * P : (kt + 1) * P],
                            rhs=qt_t[r0 : r0 + HD, qn * 512 : (qn + 1) * 512],
                            start=True,
                            stop=True,
                        )
                    nc.scalar.activation(
                        ex[:, kt % 2, :],
                        sp,
                        Exp,
                        bias=mask3_t[:, kt : kt + 1],
                        scale=1.0 / np.sqrt(HD),
                    )
                    if kt == 2 and pending_tail is not None:
                        pending_tail()
                        pending_tail = None
                    adv(2)
                    lhsT = rs.vp4[:, kt : kt + 1, h : h + 1, :]
                    for qn in range(2):
                        nc.tensor.matmul(
                            cps[0:VW, qn * 512 : (qn + 1) * 512],
                            lhsT=lhsT,
                            rhs=ex[:, kt % 2 : kt % 2 + 1, qn * 512 : (qn + 1) * 512],
                            start=(kt == 0),
                            stop=(kt == NK - 1),
                        )
                # Tail: drain cps (DVE copy, frees the single cps buffer fast);
                # transpose unnormalized ctx'^T blocks [65,128]->[128,65] so the
                # denominator becomes per-partition, then reciprocal + per-qtile
                # tensor_scalar_mul normalize during the outp drain. PE parts
                # are deferred into the next head's window.
                csb = po.csb.tile([P, S], F32, tag="csb", name="csb")
                nc.vector.tensor_copy(csb[0:VW, :], cps[0:VW, :])

                def make_tail(csb=csb, outp=outp_cur, r0=r0, hp=hp, last=(sub == 1)):
                    def tail():
                        outp3 = outp.rearrange("p (q c) -> p q c", c=P)
                        rcd = po.csb.tile([P, NS], F32, tag="rcd", name="rcd")
                        rcd3 = rcd.rearrange("p (a b) -> p a b", b=1)
                        for half in range(2):
                            tp = po.pps.tile([P, 512], F32, tag="pps", name="tpc")
                            for j in range(4):
                                qi = half * 4 + j
                                nc.tensor.transpose(
                                    tp[:, j * VW : (j + 1) * VW],
                                    csb[0:VW, qi * P : (qi + 1) * P],
                                    ident[0:VW, 0:VW],
                                )
                            tp3 = tp[:, 0 : 4 * VW].rearrange("p (a b) -> p a b", b=VW)
                            nc.vector.reciprocal(
                                rcd3[:, half * 4 : (half + 1) * 4, :],
                                tp3[:, 0:4, HD : HD + 1],
                            )
                            for j in range(4):
                                qi = half * 4 + j
                                nc.vector.tensor_scalar_mul(
                                    outp3[:, qi, r0 : r0 + HD],
                                    tp[:, j * VW : j * VW + HD],
                                    rcd[:, qi : qi + 1],
                                )
                        if last:
                            nc.gpsimd.dma_start(
                                out=out.rearrange("(q p) c -> p q c", p=P)[
                                    :, :, hp * P : (hp + 1) * P
                                ],
                                in_=outp3,
                            )

                    return tail

                if defer_tail:
                    pending_tail = make_tail()
                else:
                    make_tail()()
            if pending_tail is not None:
                pending_tail()
            pending_tail = None
            # drain any filler left (end of last rep)
            adv(10**9)


def _split_excess_waits(nc):
    """This walrus build rejects instructions with more than a couple of sem
    waits. Hoist excess waits onto injected same-engine NoOps that execute
    immediately before the overfull instruction."""
    counter = 0
    for func in nc.m.functions:
        for block in func.blocks:
            insts = block.instructions
            out = []
            changed = False
            for inst in insts:
                si = inst.sync_info
                limit = 2 if type(inst).__name__ == "InstEventSemaphore" else 1
                if si is not None and len(si.on_wait) > limit:
                    waits = list(si.on_wait)
                    for w in waits[limit:]:
                        nop = mybir.InstNoOp(
                            name=f"I-wsplit-{counter}", engine=inst.engine
                        )
                        counter += 1
                        nop.sync_info = mybir.SyncInfo(on_wait=[w], on_update=[])
                        nop.debug = inst.debug
                        out.append(nop)
                    si.on_wait = waits[:limit]
                    inst.sync_info = si
                    changed = True
                out.append(inst)
            if changed:
                block.instructions = out


_NC_CACHE = {}


def _build(split_waits=True, n_reps=1, **kw):
    global _NC_CACHE
    key = (n_reps, tuple(sorted(kw.items())))
    if split_waits and key in _NC_CACHE:
        return _NC_CACHE[key]
    _patch_tile_drain()
    nc = bass.Bass(target_bir_lowering=False, debug=False)
    hs = nc.dram_tensor("hs", [S, H], F32, kind="ExternalInput").ap()
    mask = nc.dram_tensor("mask", [S], F32, kind="ExternalInput").ap()
    wq = nc.dram_tensor("wq", [H, H], F32, kind="ExternalInput").ap()
    bq = nc.dram_tensor("bq", [H], F32, kind="ExternalInput").ap()
    wk = nc.dram_tensor("wk", [H, H], F32, kind="ExternalInput").ap()
    bk = nc.dram_tensor("bk", [H], F32, kind="ExternalInput").ap()
    wv = nc.dram_tensor("wv", [H, H], F32, kind="ExternalInput").ap()
    bv = nc.dram_tensor("bv", [H], F32, kind="ExternalInput").ap()
    out = nc.dram_tensor("out", [S, H], F32, kind="ExternalOutput").ap()
    from contextlib import ExitStack

    with tile.TileContext(nc) as tc, ExitStack() as cctx:
        consts = build_consts(nc, tc, cctx, mask, bq, bk, bv)
        build_all(nc, tc, consts, hs, wq, wk, wv, out, n_reps, **kw)
    if not split_waits:
        return nc
    _split_excess_waits(nc)
    _NC_CACHE[key] = nc
    return nc


def make_in_maps(hidden_states, attention_mask, Wq, bq, Wk, bk, Wv, bv):
    f = np.ascontiguousarray
    maps = []
    for i in range(N_CORES):
        maps.append(
            {
                "hs": f(np.asarray(hidden_states[i], dtype=np.float32)),
                "mask": f(np.asarray(attention_mask[i], dtype=np.float32).reshape(S)),
                "wq": f(np.asarray(Wq, dtype=np.float32)),
                "bq": f(np.asarray(bq, dtype=np.float32)),
                "wk": f(np.asarray(Wk, dtype=np.float32)),
                "bk": f(np.asarray(bk, dtype=np.float32)),
                "wv": f(np.asarray(Wv, dtype=np.float32)),
                "bv": f(np.asarray(bv, dtype=np.float32)),
            }
        )
    return maps


def run(in_maps, **kwargs):
    nc = _build()
    return run_bass_kernel_spmd(nc, in_maps, core_ids=list(range(N_CORES)), **kwargs)


def kernel(hidden_states, attention_mask, Wq, bq, Wk, bk, Wv, bv):
    in_maps = make_in_maps(hidden_states, attention_mask, Wq, bq, Wk, bk, Wv, bv)
    res = run(in_maps)
    return np.stack([res.results[i]["out"] for i in range(N_CORES)], axis=0)
